# revision 2
# baseline (speedup 1.0000x reference)
"""Trainium2 Bass kernel for nn_ContrastiveLoss (NT-Xent with 1-D embeddings).

Math (N = 2B = 16384, T = 0.5):
  reps = concat(emb_i[:,0], emb_j[:,0])                     (N,)
  sim[a,b] = 1 / (1 + |reps[b] - reps[a]|)
  denom[a] = sum_b exp(sim[a,b]/T) - exp(1/T)               (diag removed)
  positives sum = 2 * sum_i p_i,  p_i = 1/(1+|emb_i - emb_j|)
  loss = (1/N) * ( sum_a ln(denom[a]) - (1/T)*2*sum_i p_i )

Sharding: rows of the (N,N) similarity matrix split across 8 cores
(2048 rows each); every core holds the full reps vector. Each core emits
per-partition partials of  sum ln(denom) - 0.5*sum_i p_i  (the positive
term is computed redundantly on all 8 cores, scaled by 1/8); the host
sums the 8x128 partials and divides by N.

Per-core pipeline over (row-tile 128 x col-chunk 4096) tiles:
  ACT: u = |cols - r|          (Abs with per-partition bias -r)
  DVE: w = u + 1               (tensor_scalar, 2x mode)
  DVE: s ~= 1/w                (reciprocal_approx_fast, ~51 ULP)
  ACT: e = exp(2s), accum_out -> row-sum partial
then rowsum -> ln(rowsum - e^2) -> free-axis reduce.
"""

import numpy as np

import concourse.bass as bass
import concourse.tile as tile
from concourse import bacc, mybir
from concourse.bass_utils import run_bass_kernel_spmd

F32 = mybir.dt.float32
AF = mybir.ActivationFunctionType
AX = mybir.AxisListType

P = 128            # partitions
BATCH = 8192
N = 2 * BATCH      # 16384 rows/cols of the similarity matrix
N_CORES = 8
RPC = N // N_CORES  # 2048 rows per core
RT = RPC // P       # 16 row tiles per core
CK = 4096           # col chunk (free-dim elements per instruction)
NCH = N // CK       # 4 col chunks
E2 = float(np.exp(2.0))


def _build_body(tc, out_ap, cols_ap, nrows_ap, ei_ap, ej_ap):
    nc = tc.nc
    with tc.tile_pool(name="singles", bufs=1) as singles, \
         tc.tile_pool(name="work", bufs=3) as work:

        # Full reps vector, broadcast to all 128 partitions, in NCH chunks.
        colt = []
        for c in range(NCH):
            t = singles.tile([P, CK], F32, tag=f"col{c}")
            nc.sync.dma_start(
                out=t[:], in_=cols_ap[c * CK:(c + 1) * CK].partition_broadcast(P)
            )
            colt.append(t)

        # Negated row values for this core's block: (p, t) holds
        # -reps[core_off + p*RT + t]; any bijection rows<->(p,t) is fine
        # since we only ever sum over rows.
        nrt = singles.tile([P, RT], F32, tag="nrows")
        nc.sync.dma_start(out=nrt[:], in_=nrows_ap.rearrange("(p t) -> p t", p=P))

        # Per-(row-tile, chunk) row-sum partials.
        acc = singles.tile([P, RT * NCH], F32, tag="acc")

        for t in range(RT):
            for c in range(NCH):
                u = work.tile([P, CK], F32, tag="u")
                nc.scalar.activation(
                    u[:], colt[c][:], AF.Abs, bias=nrt[:, t:t + 1], scale=1.0
                )
                w = work.tile([P, CK], F32, tag="w")
                nc.vector.tensor_scalar_add(w[:], u[:], 1.0)
                s = work.tile([P, CK], F32, tag="u")
                nc.vector.reciprocal_approx_fast(s[:], w[:])
                e = work.tile([P, CK], F32, tag="w")
                nc.scalar.activation(
                    e[:], s[:], AF.Exp, scale=2.0,
                    accum_out=acc[:, t * NCH + c: t * NCH + c + 1],
                )

        # rowsum over chunks, then ln(rowsum - e^2), then reduce over row tiles.
        rowsum = singles.tile([P, RT], F32, tag="rowsum")
        nc.vector.reduce_sum(
            rowsum[:], acc[:].rearrange("p (t c) -> p t c", c=NCH), axis=AX.X
        )
        ne2 = singles.tile([P, 1], F32, tag="ne2")
        nc.vector.memset(ne2[:], -E2)
        logd = singles.tile([P, RT], F32, tag="logd")
        nc.scalar.activation(logd[:], rowsum[:], AF.Ln, bias=ne2[:])
        ld_red = singles.tile([P, 1], F32, tag="ld_red")
        nc.vector.reduce_sum(ld_red[:], logd[:], axis=AX.X)

        # Positive-pair term p_i = 1/(1+|emb_i - emb_j|), computed on every
        # core (scaled by 1/8 at combine time below).
        PB = BATCH // P  # 64
        eit = singles.tile([P, PB], F32, tag="eit")
        nc.sync.dma_start(out=eit[:], in_=ei_ap.rearrange("(p t) -> p t", p=P))
        ejt = singles.tile([P, PB], F32, tag="ejt")
        nc.sync.dma_start(out=ejt[:], in_=ej_ap.rearrange("(p t) -> p t", p=P))
        pd = singles.tile([P, PB], F32, tag="pd")
        nc.vector.tensor_sub(pd[:], eit[:], ejt[:])
        pu = singles.tile([P, PB], F32, tag="pu")
        nc.scalar.activation(pu[:], pd[:], AF.Abs)
        pw = singles.tile([P, PB], F32, tag="pw")
        nc.vector.tensor_scalar_add(pw[:], pu[:], 1.0)
        ps = singles.tile([P, PB], F32, tag="ps")
        nc.vector.reciprocal_approx_fast(ps[:], pw[:])
        pred = singles.tile([P, 1], F32, tag="pred")
        nc.vector.reduce_sum(pred[:], ps[:], axis=AX.X)

        # out = ld_red - 0.5 * pred   (loss*N = sum ln(denom) - 4*sum p_i,
        # and the p-term is replicated on 8 cores: 4/8 = 0.5)
        predm = singles.tile([P, 1], F32, tag="predm")
        nc.vector.tensor_scalar_mul(predm[:], pred[:], -0.5)
        outp = singles.tile([P, 1], F32, tag="outp")
        nc.vector.tensor_add(outp[:], ld_red[:], predm[:])
        nc.sync.dma_start(out=out_ap, in_=outp[:])


_NC_CACHE = None


def _get_nc():
    global _NC_CACHE
    if _NC_CACHE is None:
        nc = bacc.Bacc(
            "TRN2", target_bir_lowering=False, debug=False, num_devices=N_CORES
        )
        cols = nc.dram_tensor("cols", [N], F32, kind="ExternalInput").ap()
        nrows = nc.dram_tensor("nrows", [RPC], F32, kind="ExternalInput").ap()
        ei = nc.dram_tensor("ei", [BATCH], F32, kind="ExternalInput").ap()
        ej = nc.dram_tensor("ej", [BATCH], F32, kind="ExternalInput").ap()
        out = nc.dram_tensor("out", [P, 1], F32, kind="ExternalOutput").ap()
        with tile.TileContext(nc) as tc:
            _build_body(tc, out, cols, nrows, ei, ej)
        nc.compile()
        _NC_CACHE = nc
    return _NC_CACHE


def _make_in_maps(emb_i, emb_j):
    emb_i = np.ascontiguousarray(np.asarray(emb_i, dtype=np.float32)).reshape(BATCH)
    emb_j = np.ascontiguousarray(np.asarray(emb_j, dtype=np.float32)).reshape(BATCH)
    reps = np.concatenate([emb_i, emb_j])
    in_maps = []
    for k in range(N_CORES):
        in_maps.append({
            "cols": reps,
            "nrows": -reps[k * RPC:(k + 1) * RPC],
            "ei": emb_i,
            "ej": emb_j,
        })
    return in_maps


def _combine(results):
    total = 0.0
    for r in results:
        total += float(np.sum(r["out"], dtype=np.float64))
    return np.asarray(total / N, dtype=np.float32)


def run_spmd(emb_i, emb_j, **kwargs):
    """Run the kernel on 8 cores; returns (loss ndarray, BassKernelResults)."""
    nc = _get_nc()
    res = run_bass_kernel_spmd(
        nc, _make_in_maps(emb_i, emb_j), list(range(N_CORES)), **kwargs
    )
    return _combine(res.results), res


def kernel(emb_i, emb_j):
    loss, _ = run_spmd(emb_i, emb_j)
    return loss
